# revision 38
# baseline (speedup 1.0000x reference)
"""Causal multi-head attention (16 heads, head_dim 128, QK-RMSNorm + RoPE)
distributed over 8 Trainium2 NeuronCores.

Ulysses-style sharding (sequence-parallel -> head-parallel via AllToAll):

  P1   core c handles sequence rows 256c..256(c+1) of BOTH batches: QKV
       projections for ALL 2048 inner dims, QK-RMSNorm over the full
       inner dim (purely local), RoPE, per-head PE transposes.
       Weights arrive as 1/8 row-shards and are AllGathered on-device
       (bf16), so the host never duplicates or transposes anything.
  A2A  one 8-core AllToAll swaps sequence-sharding for (batch, head)
       sharding: afterwards core c owns batch c//4 and heads
       4(c%4)..4(c%4)+4 for the full sequence (q^T/k^T [dh, n] and v
       natural [n, dh-block]).
  P4   causal attention per (q-tile 512, head): S^T = kT^T @ qT chunks,
       exp on ACT (1/sqrt(dh) folded into q's norm scale), causal via
       skipping invisible k-chunks + one triangular mask multiply on
       diagonal blocks; PV accumulates O^T [dh, q] in PSUM; softmax
       denominator l via a ones-column matmul; 1/l partition-broadcast
       via PE outer product.
  P6   output projection with the core's Wo row-block (pair-AllGathered)
       producing partial out in NATURAL [q, d] layout; per-q-tile
       ReduceScatter(add) within the batch group hands each core the
       finished 128-row q-strips it returns to the host.

Host: contiguous slices only (x rows cast to bf16, W row-shards cast to
bf16, rope table rows); gathers per-core natural q-strips into the full
output.

The device program can be built with reps>1 (the full pipeline repeated
back-to-back inside one NEFF); the marginal time between reps=1 and
reps=2 programs measures true device execution time independent of the
per-dispatch RPC overhead.
"""

import numpy as np

B = 2
N = 2048          # sequence length
D = 2048          # model dim
H = 16            # total heads
DH = 128          # head dim
NSB = N // 8      # 256: sequence rows per core per batch (pre-A2A)
HPC = 4           # heads per core after A2A
IPC = HPC * DH    # 512 inner dims per core after A2A
KD = D // 128     # 16 contraction chunks of the model dim
NCL = 4           # local row chunks of 128 (2 per batch)
NCH = N // 128    # 16 global sequence chunks
NQT = N // 512    # 4 q tiles of 512
WSH = D // 8      # 256 weight shard rows per core
SHR = 3 * IPC     # 1536 rows per AllToAll shard (q | k | v)
ROPE_BASE = 50000.0
EPS = 1e-6
SCALE = 1.0 / np.sqrt(DH)
N_CORES = 8
GROUPS = [[0, 1, 2, 3], [4, 5, 6, 7]]
PAIRS = [[0, 4], [1, 5], [2, 6], [3, 7]]
ALL8 = [[0, 1, 2, 3, 4, 5, 6, 7]]

_cache = {}


def _build_program(apply_qn: bool, reps: int = 1, ablate: frozenset = frozenset()):
    """ablate is a timing-experiment tool: {"ag","a2a","rs"} replace the
    corresponding collectives with local stand-ins (results become wrong,
    timing stays comparable)."""
    import concourse.bass as bass
    import concourse.mybir as mybir
    import concourse.tile as tile
    from concourse import bacc

    f32 = mybir.dt.float32
    f32r = mybir.dt.float32r
    bf16 = mybir.dt.bfloat16
    AF = mybir.ActivationFunctionType
    Alu = mybir.AluOpType

    nc = bacc.Bacc("TRN2", target_bir_lowering=False, debug=False,
                   num_devices=N_CORES)

    # ---- I/O (everything a contiguous host slice) ----
    xs0 = nc.dram_tensor("xs0", [NSB, D], bf16, kind="ExternalInput").ap()
    xs1 = nc.dram_tensor("xs1", [NSB, D], bf16, kind="ExternalInput").ap()
    wqs = nc.dram_tensor("wqs", [WSH, D], bf16, kind="ExternalInput").ap()
    wks = nc.dram_tensor("wks", [WSH, D], bf16, kind="ExternalInput").ap()
    wvs = nc.dram_tensor("wvs", [WSH, D], bf16, kind="ExternalInput").ap()
    wos = nc.dram_tensor("wos", [4 * DH, D], bf16, kind="ExternalInput").ap()
    qn = nc.dram_tensor("qn", [1, D], f32, kind="ExternalInput").ap()
    kn = nc.dram_tensor("kn", [1, D], f32, kind="ExternalInput").ap()
    cos_d = nc.dram_tensor("cos", [NSB, DH], f32, kind="ExternalInput").ap()
    sin_d = nc.dram_tensor("sin_s", [NSB, DH], f32, kind="ExternalInput").ap()
    tri_d = nc.dram_tensor("tri", [128, 128], bf16, kind="ExternalInput").ap()
    idn_d = nc.dram_tensor("idn", [128, 128], f32, kind="ExternalInput").ap()
    idnb_d = nc.dram_tensor("idn_b", [128, 128], bf16,
                            kind="ExternalInput").ap()
    ones_d = nc.dram_tensor("ones_col", [128, 2], bf16,
                            kind="ExternalInput").ap()
    onesr_d = nc.dram_tensor("ones_row", [1, 128], f32r,
                             kind="ExternalInput").ap()
    # natural output q-tile: core (b, r) returns batch b rows
    # [512r : 512(r+1)]
    outn = nc.dram_tensor("outn", [512, D], bf16,
                          kind="ExternalOutput").ap()

    xs_r = {0: xs0.rearrange("(c p) d -> p c d", p=128),   # [128, 2, D]
            1: xs1.rearrange("(c p) d -> p c d", p=128)}
    cos_r = cos_d.rearrange("(c p) d -> p c d", p=128)     # [128, 2, DH]
    sin_r = sin_d.rearrange("(c p) d -> p c d", p=128)
    wsh_r = {0: wqs.rearrange("(a p) d -> p a d", p=128),
             1: wks.rearrange("(a p) d -> p a d", p=128),
             2: wvs.rearrange("(a p) d -> p a d", p=128)}  # [128, 2, D]

    def emit_rep(tc, rep):
        dram = tc.alloc_tile_pool(name=f"dram{rep}", bufs=1, space="DRAM")
        const = tc.alloc_tile_pool(name=f"const{rep}", bufs=1)
        sb = tc.alloc_tile_pool(name=f"sb{rep}", bufs=1)

        # ---------- DRAM internals ----------
        # collectives pay a ~75us floor each on this fabric, so there are
        # exactly four: one merged weight AllGather, two AllToAlls (the q
        # one hides behind k/v compute), one output ReduceScatter.
        # all exchange buffers use >=2KB rows for efficient SDMA runs.
        w_in = dram.tile([3 * WSH, D], bf16, name=f"w_in_{rep}")
        w_all = dram.tile([3 * D, D], bf16, name=f"w_all_{rep}",
                          addr_space="Shared")
        # q exchange: shard j = [128 dh-rows, (4 h, 2 u, 128 n)]
        a2a_q_in = dram.tile([8 * 128, 1024], bf16, name=f"a2a_qi_{rep}")
        a2a_q_out = dram.tile([8 * 128, 1024], bf16, name=f"a2a_qo_{rep}")
        # kv exchange: shard j = [k^T 128 rows like q | v 128 rows of
        # (2 u, 512 ic) natural]
        a2a_kv_in = dram.tile([8 * 256, 1024], bf16, name=f"a2a_kvi_{rep}")
        a2a_kv_out = dram.tile([8 * 256, 1024], bf16, name=f"a2a_kvo_{rep}")
        rs_in = dram.tile([N, D], bf16, name=f"rs_in_{rep}")
        rs_out = dram.tile([512, D], bf16, name=f"rs_out_{rep}")

        # ---------- constants ----------
        tri = const.tile([128, 128], bf16, tag="tri", name=f"tri_sb{rep}")
        idn_b = const.tile([128, 128], bf16, tag="idnb", name=f"idnb{rep}")
        ones_col = const.tile([128, 2], bf16, tag="ones", name=f"ones{rep}")
        ones_row = const.tile([1, 128], f32r, tag="ones_r",
                              name=f"ones_r{rep}")
        eps_t = const.tile([128, 1], f32, tag="eps", name=f"eps_t{rep}")
        nc.gpsimd.memset(eps_t[:], EPS)
        cos_sb = const.tile([128, 2, DH], f32, tag="cos", name=f"cos{rep}")
        sin_sb = const.tile([128, 2, DH], f32, tag="sin", name=f"sin{rep}")
        nc.scalar.dma_start(cos_sb[:], cos_r)
        nc.scalar.dma_start(sin_sb[:], sin_r)
        nc.gpsimd.dma_start(tri[:], tri_d)
        nc.gpsimd.dma_start(idn_b[:], idnb_d)
        nc.gpsimd.dma_start(ones_col[:], ones_d)
        nc.gpsimd.dma_start(ones_row[:], onesr_d)
        if apply_qn:
            qn_b = const.tile([128, D], f32, tag="qn_b", name=f"qn_b{rep}")
            nc.gpsimd.dma_start(qn_b[:], qn.to_broadcast((128, D)))
            kn_b = const.tile([128, D], f32, tag="kn_b", name=f"kn_b{rep}")
            nc.gpsimd.dma_start(kn_b[:], kn.to_broadcast((128, D)))

        # q/k/v weight shards bounce SBUF -> one packed internal DRAM
        # tile [wq_c | wk_c | wv_c], then ONE AllGather
        wbp = tc.alloc_tile_pool(name=f"wbp{rep}", bufs=1)
        for t in (0, 1, 2):
            wb = wbp.tile([128, 2, D], bf16, tag="wb", bufs=2,
                          name=f"wb{t}_{rep}")
            nc.scalar.dma_start(wb[:], wsh_r[t])
            nc.scalar.dma_start(
                w_in[WSH * t:WSH * (t + 1), :].rearrange(
                    "(a p) d -> p a d", p=128), wb[:])
        if "ag" not in ablate:
            nc.gpsimd.collective_compute(
                "AllGather", Alu.bypass, replica_groups=ALL8,
                ins=[w_in[:].opt()], outs=[w_all[:].opt()])

        def wchunk(t, dk):
            # W_t rows [128*dk : 128*(dk+1)] inside the gathered pack:
            # shard c = rows [768c : 768(c+1)] = [wq_c | wk_c | wv_c]
            c, e = divmod(dk, 2)
            r0 = 768 * c + 256 * t + 128 * e
            return w_all[r0:r0 + 128, :]

        # ---------- local x -> x^T (PE transposes) ----------
        x_pool = tc.alloc_tile_pool(name=f"x_pool{rep}", bufs=1)
        x_sb = x_pool.tile([128, NCL, D], bf16, tag="x_sb", name=f"x_sb{rep}")
        nc.sync.dma_start(x_sb[:, 0:2, :], xs_r[0])
        nc.sync.dma_start(x_sb[:, 2:4, :], xs_r[1])
        xT_sb = sb.tile([128, KD, 512], bf16, tag="xT", name=f"xT_sb{rep}")
        psX = tc.alloc_tile_pool(name=f"psX{rep}", bufs=1, space="PSUM")
        for nci in range(NCL):
            for dk in range(KD):
                psx = psX.tile([128, 128], bf16, tag="ptx", bufs=4,
                               name=f"ptx{nci}_{dk}_{rep}")
                nc.tensor.transpose(
                    psx[:], x_sb[:, nci, dk * 128:(dk + 1) * 128], idn_b[:])
                nc.scalar.copy(
                    xT_sb[:, dk, nci * 128:(nci + 1) * 128], psx[:])
        psX.release()
        x_pool.release()
        wbp.release()

        # ---------- P1: QKV projections + norm + rope + transposes ----
        p1 = tc.alloc_tile_pool(name=f"p1_{rep}", bufs=2)
        mid = tc.alloc_tile_pool(name=f"mid{rep}", bufs=1)
        psA = tc.alloc_tile_pool(name=f"psA{rep}", bufs=1, space="PSUM")
        psT = tc.alloc_tile_pool(name=f"psT{rep}", bufs=1, space="PSUM")

        # per-core sums of squares: column nci*4+qu (row chunks are
        # distinct 128-row groups that share partitions!)
        ssq = {0: sb.tile([128, 16], f32, tag="ssq_q", name=f"ssq_q{rep}"),
               1: sb.tile([128, 16], f32, tag="ssq_k", name=f"ssq_k{rep}")}
        rr = {}   # 1/rms per (t, nci) [128,1] (q also folds 1/sqrt(dh))
        # roped q/k parked in natural layout [128 rows, 16 heads, 128]
        rp = {(t, nci): mid.tile([128, H, DH], f32, tag=f"rp{t}_{nci}",
                                 name=f"rp{t}_{nci}_{rep}")
              for t in range(2) for nci in range(NCL)}
        # scaled bf16 copies (transposed at 1 cyc/row on PE)
        rpb = {(t, nci): mid.tile([128, H, DH], bf16, tag=f"rpb{t}_{nci}",
                                  name=f"rpb{t}_{nci}_{rep}")
               for t in range(2) for nci in range(NCL)}
        # per-head transposed q/k for the A2A [128 dh, head, 4 nci, 128]
        qkT_loc = [mid.tile([128, H, NCL, 128], bf16, tag=f"qkT{t}",
                            name=f"qkT{t}_{rep}") for t in range(2)]

        def p1_phase(t):
            # t: 0=q, 1=k, 2=v
            for qu in range(4):           # inner-dim quarter (4 heads)
                pss = []
                for nci in range(NCL):
                    ps = psA.tile([128, 512], f32, tag="p1", bufs=6,
                                  name=f"p1_{t}_{qu}_{nci}_{rep}")
                    pss.append(ps)
                for dk in range(KD):
                    wch = p1.tile([128, 512], bf16, tag="wch", bufs=10,
                                  name=f"wch{t}_{qu}_{dk}_{rep}")
                    nc.sync.dma_start(
                        wch[:],
                        wchunk(t, dk)[:, qu * 512:(qu + 1) * 512])
                    for nci in range(NCL):
                        nc.tensor.matmul(
                            pss[nci][:],
                            xT_sb[:, dk, nci * 128:(nci + 1) * 128],
                            wch[:], start=(dk == 0), stop=(dk == KD - 1))
                for nci in range(NCL):
                    ps = pss[nci]
                    bb, u = divmod(nci, 2)
                    if t == 2:
                        # v natural bf16, straight into the kv tile:
                        # shard j rows [256j+128 : 256j+256], cols (u, ic)
                        vst = p1.tile([128, 512], bf16, tag="vst",
                                      bufs=4, name=f"vst{qu}_{nci}_{rep}")
                        nc.vector.tensor_copy(vst[:], ps[:])
                        j = 4 * bb + qu
                        r0 = 256 * j + 128
                        nc.scalar.dma_start(
                            a2a_kv_in[r0:r0 + 128, 512 * u:512 * (u + 1)],
                            vst[:])
                        continue
                    nc.scalar.activation(
                        p1.tile([128, 512], f32, tag="sqs", bufs=2,
                                name=f"sqs{t}_{qu}_{nci}_{rep}")[:],
                        ps[:], AF.Square,
                        accum_out=ssq[t][:, 4 * nci + qu:4 * nci + qu + 1])
                    src = ps[:].rearrange("p (h d) -> p h d", h=4)
                    if apply_qn:
                        wn = (qn_b if t == 0 else kn_b)[
                            :, qu * 512:(qu + 1) * 512].rearrange(
                                "p (h d) -> p h d", h=4)
                        wev = p1.tile([128, 4, DH], f32, tag="wev",
                                      bufs=2, name=f"wev{t}{qu}{nci}_{rep}")
                        nc.vector.tensor_mul(wev[:], src, wn)
                        src = wev[:]
                    cos_bc = cos_sb[:, u:u + 1, :].to_broadcast(
                        (128, 4, DH))
                    t1 = p1.tile([128, 4, DH], f32, tag="t1", bufs=2,
                                 name=f"t1_{t}{qu}{nci}_{rep}")
                    nc.vector.tensor_mul(t1[:], src, cos_bc)
                    t2 = p1.tile([128, 4, DH], f32, tag="t2", bufs=2,
                                 name=f"t2_{t}{qu}{nci}_{rep}")
                    nc.vector.tensor_mul(
                        t2[:, :, 0:64], src[:, :, 64:128],
                        sin_sb[:, u:u + 1, 0:64].to_broadcast((128, 4, 64)))
                    nc.vector.tensor_mul(
                        t2[:, :, 64:128], src[:, :, 0:64],
                        sin_sb[:, u:u + 1, 64:128].to_broadcast((128, 4, 64)))
                    nc.vector.tensor_add(
                        rp[(t, nci)][:, 4 * qu:4 * qu + 4, :],
                        t1[:], t2[:])

        def rms_chain(t):
            for nci in range(NCL):
                c0 = 4 * nci
                s01 = sb.tile([128, 1], f32, tag=f"s01_{t}{nci}",
                              name=f"s01_{t}{nci}_{rep}")
                nc.vector.tensor_add(s01[:], ssq[t][:, c0:c0 + 1],
                                     ssq[t][:, c0 + 1:c0 + 2])
                s23 = sb.tile([128, 1], f32, tag=f"s23_{t}{nci}",
                              name=f"s23_{t}{nci}_{rep}")
                nc.vector.tensor_add(s23[:], ssq[t][:, c0 + 2:c0 + 3],
                                     ssq[t][:, c0 + 3:c0 + 4])
                sall = sb.tile([128, 1], f32, tag=f"sa_{t}{nci}",
                               name=f"sa_{t}{nci}_{rep}")
                nc.vector.tensor_add(sall[:], s01[:], s23[:])
                rms = sb.tile([128, 1], f32, tag=f"rms_{t}{nci}",
                              name=f"rms_{t}{nci}_{rep}")
                nc.scalar.activation(rms[:], sall[:], AF.Sqrt,
                                     scale=1.0 / D, bias=eps_t[:])
                rr_t = sb.tile([128, 1], f32, tag=f"rr_{t}{nci}",
                               name=f"rr_{t}{nci}_{rep}")
                nc.vector.reciprocal(rr_t[:], rms[:])
                if t == 0:
                    # fold the SDPA 1/sqrt(dh) into q's norm scale
                    nc.vector.tensor_scalar_mul(rr_t[:], rr_t[:], SCALE)
                rr[(t, nci)] = rr_t

        def scale_transpose(t):
            # apply 1/rms (-> bf16), then per-head PE transposes into
            # the packed per-(head, nci) layout
            for nci in range(NCL):
                nc.vector.tensor_scalar_mul(rpb[(t, nci)][:],
                                            rp[(t, nci)][:],
                                            rr[(t, nci)][:, 0:1])
                for j in range(4):
                    pst = psT.tile([128, 512], bf16, tag="pst", bufs=2,
                                   name=f"pst{t}_{nci}_{j}_{rep}")
                    for hh in range(4):
                        nc.tensor.transpose(
                            pst[:, hh * 128:(hh + 1) * 128],
                            rpb[(t, nci)][:, 4 * j + hh, :], idn_b[:])
                    nc.scalar.copy(
                        qkT_loc[t][:, 4 * j:4 * j + 4, nci, :],
                        pst[:].rearrange("p (h n) -> p h n", h=4))

        def pack_qk(t):
            # shard j=(bb,g): [128 dh-rows, (4 h, 2 u, 128 n)] where
            # (u, n) are the row chunks of batch bb
            for bb in range(2):
                for g in range(4):
                    j = 4 * bb + g
                    if t == 1:
                        dst = a2a_kv_in[256 * j:256 * j + 128, :]
                    else:
                        dst = a2a_q_in[128 * j:128 * (j + 1), :]
                    dst = dst.rearrange("p (h u n) -> p h u n", h=4, u=2)
                    nc.scalar.dma_start(
                        dst,
                        qkT_loc[t][:, 4 * g:4 * g + 4, 2 * bb:2 * bb + 2, :])

        p1_phase(0)            # q matmuls first
        rms_chain(0)
        scale_transpose(0)
        pack_qk(0)
        if "a2a" not in ablate:
            nc.gpsimd.collective_compute(
                "AllToAll", Alu.bypass, replica_groups=ALL8,
                ins=[a2a_q_in[:].opt()], outs=[a2a_q_out[:].opt()])

        p1_phase(1)            # k matmuls (overlap the q exchange)
        rms_chain(1)
        scale_transpose(1)
        pack_qk(1)
        p1_phase(2)            # v matmuls (straight into the kv tile)
        if "a2a" not in ablate:
            nc.gpsimd.collective_compute(
                "AllToAll", Alu.bypass, replica_groups=ALL8,
                ins=[a2a_kv_in[:].opt()], outs=[a2a_kv_out[:].opt()])
        else:
            a2a_kv_out, a2a_q_out = a2a_kv_in, a2a_q_in

        mid.release()
        p1.release()
        psT.release()
        psA.release()

        # ---------- assemble attention operands ----------
        late = tc.alloc_tile_pool(name=f"late{rep}", bufs=1)
        qT = [sb.tile([128, N], bf16, tag=f"qT{h}", name=f"qT{h}_{rep}")
              for h in range(HPC)]
        kT = [sb.tile([128, N], bf16, tag=f"kT{h}", name=f"kT{h}_{rep}")
              for h in range(HPC)]
        v_tiles = [late.tile([128, IPC], bf16, tag=f"v{i}",
                             name=f"v{i}_{rep}") for i in range(NCH)]
        # emit in rough consumption order (h0's k/q first), split across
        # the two DMA queues
        for h in range(HPC):
            for i in range(8):
                nc.sync.dma_start(
                    kT[h][:, NSB * i:NSB * (i + 1)],
                    a2a_kv_out[256 * i:256 * i + 128,
                               256 * h:256 * (h + 1)])
                nc.scalar.dma_start(
                    qT[h][:, NSB * i:NSB * (i + 1)],
                    a2a_q_out[128 * i:128 * (i + 1),
                              256 * h:256 * (h + 1)])
            if h == 0:
                for kc in range(NCH):
                    i, u = divmod(kc, 2)
                    nc.sync.dma_start(
                        v_tiles[kc][:],
                        a2a_kv_out[256 * i + 128:256 * i + 256,
                                   512 * u:512 * (u + 1)])
        wo_sb = late.tile([128, HPC, D], bf16, tag="wo_sb", name=f"wo{rep}")
        nc.sync.dma_start(
            wo_sb[:], wos.rearrange("(io p) m -> p io m", p=128))

        # ---------- P4 attention + interleaved P6 ----------
        p4 = tc.alloc_tile_pool(name=f"p4_{rep}", bufs=1)
        psB = tc.alloc_tile_pool(name=f"psB{rep}", bufs=1, space="PSUM")
        for qt in range(NQT):
            qsl = slice(qt * 512, (qt + 1) * 512)
            o_tiles = []
            for h in range(HPC):
                n_kc = 4 * (qt + 1)
                ps_o = psB.tile([128, 512], f32, tag="ps_o", bufs=2,
                                name=f"pso{qt}_{h}_{rep}")
                ps_l = psB.tile([2, 512], f32, tag="ps_l", bufs=1,
                                name=f"psl{qt}_{h}_{rep}")
                for kc in range(n_kc):
                    ps_s = psB.tile([128, 512], f32, tag="ps_a",
                                    bufs=3, name=f"pss{qt}_{h}_{kc}_{rep}")
                    nc.tensor.matmul(
                        ps_s[:],
                        kT[h][:, kc * 128:(kc + 1) * 128],
                        qT[h][:, qsl],
                        start=True, stop=True)
                    pT = p4.tile([128, 512], bf16, tag="pT",
                                 name=f"pT{qt}_{h}_{kc}_{rep}", bufs=4)
                    j = kc - 4 * qt
                    if j < 0:
                        nc.scalar.activation(pT[:], ps_s[:], AF.Exp)
                        j = 0
                    else:
                        nc.scalar.activation(
                            pT[:, j * 128:], ps_s[:, j * 128:], AF.Exp)
                        nc.vector.tensor_mul(
                            pT[:, j * 128:(j + 1) * 128],
                            pT[:, j * 128:(j + 1) * 128], tri[:])
                    st = kc == 0
                    sp = kc == n_kc - 1
                    nc.tensor.matmul(
                        ps_o[:, j * 128:],
                        v_tiles[kc][:, h * 128:(h + 1) * 128],
                        pT[:, j * 128:], start=st, stop=sp)
                    nc.tensor.matmul(ps_l[:, j * 128:], ones_col[:],
                                     pT[:, j * 128:],
                                     start=st, stop=sp)
                # 1/l, partition-broadcast via PE outer product
                rl = p4.tile([1, 512], f32r, tag="rl",
                             name=f"rl{qt}_{h}_{rep}", bufs=2)
                with nc.allow_low_precision(
                        reason="f32r out is bit-identical f32"):
                    nc.vector.reciprocal(rl[:], ps_l[0:1, :])
                ps_lb = psB.tile([128, 512], f32, tag="ps_a", bufs=3,
                                 name=f"ps_lb{qt}_{h}_{rep}")
                nc.tensor.matmul(ps_lb[:], ones_row[:], rl[:],
                                 start=True, stop=True)
                rlb = p4.tile([128, 512], f32, tag="rlb",
                              name=f"rlb{qt}_{h}_{rep}", bufs=2)
                nc.vector.tensor_copy(rlb[:], ps_lb[:])
                o_t = p4.tile([128, 512], bf16, tag="o_t",
                              name=f"o{qt}_{h}_{rep}", bufs=6)
                nc.vector.tensor_mul(o_t[:], ps_o[:], rlb[:])
                o_tiles.append(o_t)

            # P6: natural partial out rows [512qt : 512qt+512]
            for qb in range(4):
                for quo in range(4):
                    ps_f = psB.tile([128, 512], f32, tag="ps_f",
                                    bufs=2, name=f"psf{qt}{qb}{quo}_{rep}")
                    for h in range(HPC):
                        nc.tensor.matmul(
                            ps_f[:],
                            o_tiles[h][:, qb * 128:(qb + 1) * 128],
                            wo_sb[:, h, quo * 512:(quo + 1) * 512],
                            start=(h == 0), stop=(h == HPC - 1))
                    fev = p4.tile([128, 512], bf16, tag="fev",
                                  name=f"fev{qt}{qb}{quo}_{rep}", bufs=4)
                    nc.vector.tensor_copy(fev[:], ps_f[:])
                    nc.scalar.dma_start(
                        rs_in[512 * qt + 128 * qb:512 * qt + 128 * (qb + 1),
                              quo * 512:(quo + 1) * 512],
                        fev[:])

        # ONE ReduceScatter: rank r of the batch group gets q-tile r
        if "rs" in ablate:
            nc.sync.dma_start(outn, rs_in[0:512, :])
        else:
            nc.gpsimd.collective_compute(
                "ReduceScatter", Alu.add, replica_groups=GROUPS,
                ins=[rs_in[:].opt()], outs=[rs_out[:].opt()])
            nc.sync.dma_start(outn, rs_out[:])

        psB.release()
        p4.release()
        late.release()
        sb.release()
        const.release()
        dram.release()

    with tile.TileContext(nc) as tc:
        for rep in range(reps):
            emit_rep(tc, rep)

    nc.compile()
    return nc


def _get_program(apply_qn: bool, reps: int = 1,
                 ablate: frozenset = frozenset()):
    key = ("prog", apply_qn, reps, ablate)
    if key not in _cache:
        _cache[key] = _build_program(apply_qn, reps, ablate)
    return _cache[key]


def _rope_tables():
    inv_freq = (1.0 / (ROPE_BASE ** (np.arange(0, DH, 2, dtype=np.float32)
                                     / DH))).astype(np.float32)
    t = np.arange(N, dtype=np.float32)
    freqs = np.outer(t, inv_freq).astype(np.float32)       # [N, DH/2]
    emb = np.concatenate([freqs, freqs], axis=-1)          # [N, DH]
    cos = np.cos(emb).astype(np.float32)
    sin = np.sin(emb).astype(np.float32)
    sin_s = sin.copy()
    sin_s[:, 0:DH // 2] *= -1.0
    return cos, sin_s


_host_cache = {}


def _host_consts():
    if "consts" not in _host_cache:
        import ml_dtypes
        bf = ml_dtypes.bfloat16
        cos, sin_s = _rope_tables()
        _host_cache["consts"] = {
            "cos_t": cos, "sin_t": sin_s,
            "tri": np.triu(np.ones((128, 128), dtype=np.float32)).astype(bf),
            "idn": np.eye(128, dtype=np.float32),
            "idn_b": np.eye(128, dtype=np.float32).astype(bf),
            "ones_col": np.ones((128, 2), dtype=bf),
            "ones_row": np.ones((1, 128), dtype=np.float32),
        }
    return _host_cache["consts"]


def make_in_maps(x, Wq, Wk, Wv, Wo, qn_w, kn_w):
    import ml_dtypes
    bf = ml_dtypes.bfloat16
    c_ = _host_consts()
    x_b = x.astype(bf)
    Wq_b = Wq.astype(bf)
    Wk_b = Wk.astype(bf)
    Wv_b = Wv.astype(bf)
    Wo_b = Wo.astype(bf)
    qn2 = np.ascontiguousarray(qn_w).reshape(1, D)
    kn2 = np.ascontiguousarray(kn_w).reshape(1, D)
    in_maps = []
    for c in range(N_CORES):
        b, g = divmod(c, 4)
        in_maps.append({
            "xs0": x_b[0, NSB * c:NSB * (c + 1), :],
            "xs1": x_b[1, NSB * c:NSB * (c + 1), :],
            "wqs": Wq_b[WSH * c:WSH * (c + 1)],
            "wks": Wk_b[WSH * c:WSH * (c + 1)],
            "wvs": Wv_b[WSH * c:WSH * (c + 1)],
            # the core's full Wo row block (heads 4g..4g+4)
            "wos": Wo_b[512 * g:512 * (g + 1)],
            "qn": qn2, "kn": kn2,
            "cos": c_["cos_t"][NSB * c:NSB * (c + 1)],
            "sin_s": c_["sin_t"][NSB * c:NSB * (c + 1)],
            "tri": c_["tri"], "idn": c_["idn"], "idn_b": c_["idn_b"],
            "ones_col": c_["ones_col"], "ones_row": c_["ones_row"],
        })
    return in_maps


def assemble_output(results):
    out = np.empty((B, N, D), dtype=np.float32)
    for c in range(N_CORES):
        b, r = divmod(c, 4)
        out[b, 512 * r:512 * (r + 1), :] = np.asarray(
            results[c]["outn"]).astype(np.float32)
    return out


def _get_runner(apply_qn: bool, reps: int = 1,
                ablate: frozenset = frozenset()):
    """Build (once) a cached jitted PJRT runner for the 8-core program."""
    key = ("runner", apply_qn, reps, ablate)
    if key in _cache:
        return _cache[key]

    import jax
    from jax.sharding import Mesh, PartitionSpec
    try:
        from jax.experimental.shard_map import shard_map
    except ImportError:
        from jax.shard_map import shard_map
    import concourse.mybir as mybir
    from concourse.bass2jax import (_bass_exec_p, install_neuronx_cc_hook,
                                    partition_id_tensor)

    nc = _get_program(apply_qn, reps, ablate)
    install_neuronx_cc_hook()

    partition_name = (nc.partition_id_tensor.name
                      if nc.partition_id_tensor else None)
    in_names, out_names, out_avals = [], [], []
    for alloc in nc.m.functions[0].allocations:
        if not isinstance(alloc, mybir.MemoryLocationSet):
            continue
        name = alloc.memorylocations[0].name
        if alloc.kind == "ExternalInput":
            if name != partition_name:
                in_names.append(name)
        elif alloc.kind == "ExternalOutput":
            shape = tuple(alloc.tensor_shape)
            dtype = mybir.dt.np(alloc.dtype)
            out_names.append(name)
            out_avals.append(jax.core.ShapedArray(shape, dtype))
    n_params = len(in_names)
    n_outs = len(out_names)
    all_in_names = in_names + out_names
    if partition_name is not None:
        all_in_names = all_in_names + [partition_name]
    donate = tuple(range(n_params, n_params + n_outs))

    def _body(*args):
        operands = list(args)
        if partition_name is not None:
            operands.append(partition_id_tensor())
        outs = _bass_exec_p.bind(
            *operands,
            out_avals=tuple(out_avals),
            in_names=tuple(all_in_names),
            out_names=tuple(out_names),
            lowering_input_output_aliases=(),
            sim_require_finite=True,
            sim_require_nnan=True,
            nc=nc,
        )
        return tuple(outs)

    devices = jax.devices()[:N_CORES]
    mesh = Mesh(np.asarray(devices), ("core",))
    in_specs = (PartitionSpec("core"),) * (n_params + n_outs)
    out_specs = (PartitionSpec("core"),) * n_outs
    fn = jax.jit(
        shard_map(_body, mesh=mesh, in_specs=in_specs, out_specs=out_specs,
                  check_rep=False),
        donate_argnums=donate, keep_unused=True)

    import jax.numpy as jnp
    from jax.sharding import NamedSharding
    zero_shardings = [NamedSharding(mesh, PartitionSpec("core"))] * n_outs
    zero_shapes = [(N_CORES * a.shape[0], *a.shape[1:]) for a in out_avals]
    zero_dtypes = [a.dtype for a in out_avals]

    def make_zeros():
        return [jax.device_put(jnp.zeros(s, d), sh)
                for s, d, sh in zip(zero_shapes, zero_dtypes, zero_shardings)]

    runner = {
        "fn": fn, "in_names": in_names, "out_names": out_names,
        "out_avals": out_avals, "make_zeros": make_zeros, "mesh": mesh,
    }
    _cache[key] = runner
    return runner


def _concat_inputs(runner, in_maps):
    return [np.concatenate([in_maps[c][name] for c in range(N_CORES)], axis=0)
            for name in runner["in_names"]]


def _run(runner, concat_in):
    out_arrs = runner["fn"](*concat_in, *runner["make_zeros"]())
    res = []
    for c in range(N_CORES):
        res.append({
            name: np.asarray(out_arrs[i]).reshape(
                N_CORES, *runner["out_avals"][i].shape)[c]
            for i, name in enumerate(runner["out_names"])})
    return res


def kernel(x, Wq, Wk, Wv, Wo, qn_w, kn_w):
    x = np.asarray(x, dtype=np.float32)
    Wq = np.asarray(Wq, dtype=np.float32)
    Wk = np.asarray(Wk, dtype=np.float32)
    Wv = np.asarray(Wv, dtype=np.float32)
    Wo = np.asarray(Wo, dtype=np.float32)
    qn_w = np.asarray(qn_w, dtype=np.float32)
    kn_w = np.asarray(kn_w, dtype=np.float32)

    apply_qn = not (np.all(qn_w == 1.0) and np.all(kn_w == 1.0))
    runner = _get_runner(apply_qn)
    in_maps = make_in_maps(x, Wq, Wk, Wv, Wo, qn_w, kn_w)
    res = _run(runner, _concat_inputs(runner, in_maps))
    return assemble_output(res)
